# revision 28
# baseline (speedup 1.0000x reference)
"""GroupedQueryAttention (B=2, N=2048, D=2048, H=16, HKV=4, HD=128) on 8 trn2 cores.

Sharding: core c handles (batch b = c//4, kv-head g = c%4): 4 q-heads + 1 kv head.
RoPE (with the reference's sin==cos quirk) is folded into Wq/Wk host-side,
so on-device RoPE is an elementwise multiply by a precomputed cos table.
Per-head AllGather of normalized attention outputs across the 4 cores of a
batch, then each core accumulates its 512 Wo columns over all 16 heads in a
transposed [d, n] fp32 SBUF accumulator (host transposes back).

Performance structure (vs the 467us v1 baseline; now ~370us):
  - Softmax-denominator matmuls run in fp8e5 + DoubleRow perf mode (two
    m-tiles contracted per pass at 0.5 cyc/row): ~4x less PE time than the
    bf16 ones-matmul. exp carries a -2*ln2 bias (applied on the ScalarE
    activation) so exp values stay well under the e5m2 max; the scale cancels
    exactly in the P/sum ratio. AV stays bf16 (fp8 P measurably breaks the
    2e-2 error budget; the all-positive denominator sum averages fp8 noise
    away).
  - exp runs on 1024-wide PSUM score pairs (two m-tiles per ACTIVATE),
    halving ScalarE instruction overhead; a DVE copy casts each pair to fp8
    for the denominator matmul.
  - Attention is one flat software pipeline per head over all 32 (ncx, pair)
    steps with scores emitted 2 pairs ahead of AV, so the PE has no bubble at
    group boundaries (PSUM: 3x[128,1024] scores ring + ot + sums = 8 banks).
  - Emission order starts attention h0 right after K, V, q0 projections;
    head h's attention interleaves head h+1's q projection (2 matmuls per
    pair), giving the PE independent work while the exp->cast chain settles.
    This launches the AllGather stream ~50us earlier (the 4 gathers are the
    near-critical path once attention is fast). Head 3's gather is split in
    two n-halves issued mid-attention so the last slab never stalls on it.
  - All inputs are host-prepacked partition-major so DMA lines are 8-32KB
    (~270GB/s vs ~170GB/s for row-major views); x arrives as two n-halves
    and K proj paces with the first half's sub-chunks.
  - Final out DMAs issue per-128-row chunk as the last slab completes them,
    alternating the two HWDGE queues (SP/Activation).
"""

import math
import sys
import types

import numpy as np

B, N, D = 2, 2048, 2048
H, HKV, HD = 16, 4, 128
G = H // HKV  # q heads per kv head = 4
N_CORES = 8
ROPE_BASE = 10000.0
DSLICE = D // G  # 512 output columns per core
JL = G * HD  # 512 local attention-output rows per core
EXP_BIAS = -2.0 * math.log(2.0)  # scales all exp values by 1/4 (cancels in ratio)


def _install_axon_ntff_hook():
    """This container's antenv lacks axon_hooks; inject it so trace=True works."""
    if "antenv.axon_hooks" in sys.modules:
        return
    try:
        from trn_agent_boot.trn_boot import _ntff_profile_via_ctypes

        hook = _ntff_profile_via_ctypes("/opt/axon/libaxon_pjrt.so")
    except Exception:
        hook = None
    mod = types.ModuleType("antenv.axon_hooks")
    mod.get_axon_ntff_profile_hook = lambda: hook
    mod.set_axon_ntff_profile_hook = lambda h: None
    sys.modules["antenv.axon_hooks"] = mod


def _fold_rope(w: np.ndarray, n_heads: int) -> np.ndarray:
    """Return W' with the (sin==cos) RoPE mixing folded in: x@W' = M(x@W) per head."""
    wf = w.reshape(D, n_heads, HD)
    lo, hi = wf[..., : HD // 2], wf[..., HD // 2 :]
    return np.concatenate([lo - hi, hi + lo], axis=-1).reshape(D, n_heads * HD)


def _cos_table() -> np.ndarray:
    inv_freq = 1.0 / (ROPE_BASE ** (np.arange(0, HD, 2, dtype=np.float64) / HD))
    freqs = np.arange(N, dtype=np.float64)[:, None] * inv_freq[None, :]  # [N, 64]
    emb = np.concatenate([freqs, freqs], axis=-1)  # [N, 128]
    return np.cos(emb).T.astype(np.float32).copy()  # [128, N]


_NC_CACHE: dict = {}


def _build_nc():
    if "nc" in _NC_CACHE:
        return _NC_CACHE["nc"]

    import concourse.bacc as bacc
    import concourse.mybir as mybir
    import concourse.tile as tile
    from concourse.bass import ts
    from concourse.masks import make_identity

    f32 = mybir.dt.float32
    bf16 = mybir.dt.bfloat16
    f8e5 = mybir.dt.float8e5
    AFT = mybir.ActivationFunctionType
    KD = D // 128  # 16 contraction chunks
    NT = N // 128  # 16 m tiles of 128
    NC512 = N // 512  # 4 n chunks of 512
    NPAIR = NT // 2  # 8 m-tile pairs
    DC = DSLICE // 128  # 4 output-row chunks of 128

    nc = bacc.Bacc(target_bir_lowering=False, debug=False, num_devices=N_CORES)

    # all inputs host-prepacked partition-major so DMA lines are 8-32KB
    # (row-major [D, cols] views only give 1-2KB lines and ~170GB/s; packed
    # transfers sustain ~270+GB/s). x comes as two n-halves so K/V/q0
    # projections of the first 1024 columns start ~20us earlier.
    xh0 = nc.dram_tensor("xh0", [128, KD, 1024], bf16, kind="ExternalInput")
    xh1 = nc.dram_tensor("xh1", [128, KD, 1024], bf16, kind="ExternalInput")
    wq = nc.dram_tensor("wq", [128, KD, JL], bf16, kind="ExternalInput")
    wk = nc.dram_tensor("wk", [128, KD, HD], bf16, kind="ExternalInput")
    wv = nc.dram_tensor("wv", [128, KD, HD], bf16, kind="ExternalInput")
    wo = nc.dram_tensor("wo", [128, KD, DSLICE], bf16, kind="ExternalInput")
    cost = nc.dram_tensor("cost", [HD, N], f32, kind="ExternalInput")
    # transposed output: outT[d, n]; host transposes back
    out = nc.dram_tensor("out", [DSLICE, N], f32, kind="ExternalOutput")

    with tile.TileContext(nc) as tc:
        with (
            tc.tile_pool(name="big", bufs=1) as big_pool,
            tc.tile_pool(name="ag", bufs=3) as ag_pool,
            tc.tile_pool(name="otn", bufs=6) as otn_pool,
            tc.tile_pool(name="wpool", bufs=1) as w_pool,
            tc.tile_pool(name="work", bufs=1) as work_pool,
            tc.tile_pool(name="st", bufs=3) as st_pool,
            tc.tile_pool(name="ev", bufs=2) as ev_pool,
            tc.tile_pool(name="psmm", bufs=2, space="PSUM") as ps_pool,
            tc.tile_pool(name="psacc", bufs=1, space="PSUM") as psacc_pool,
            tc.tile_pool(name="dram", bufs=1, space="DRAM") as dram_pool,
        ):
            # ---- persistent SBUF tensors ----
            x_sb = big_pool.tile([128, 2, KD, 1024], bf16, tag="big")
            wq_sb = w_pool.tile([128, KD, JL], bf16, tag="wq")
            wk_sb = w_pool.tile([128, KD, HD], bf16, tag="wk")
            wv_sb = w_pool.tile([128, KD, HD], bf16, tag="wv")
            wo_sb = w_pool.tile([128, KD, DSLICE], bf16, tag="wo")
            cos_sb = w_pool.tile([128, N], f32, tag="cos")
            qT_sb = work_pool.tile([128, G, N], bf16, tag="qT")
            kT_sb = work_pool.tile([128, N], bf16, tag="kT")
            vT_sb = ag_pool.tile([128, HKV, N], bf16, tag="agsb", name="vT_sb")[:, 0, :]
            v_sb = work_pool.tile([128, N], bf16, tag="v")  # [m-part, mt*128+hd]
            ones8 = work_pool.tile([128, 2, 128], f8e5, tag="ones8")
            ident_sb = work_pool.tile([128, 128], bf16, tag="ident")
            ebias_sb = work_pool.tile([128, 1], f32, tag="ebias")

            nc.gpsimd.memset(ones8[:], 1.0)
            nc.gpsimd.memset(ebias_sb[:], EXP_BIAS)
            make_identity(nc, ident_sb[:])

            # ---- input DMAs: weights for K/V first, then x half 0 in 4-kd
            # sub-chunks (K p0 starts paced with them), then wq, x half 1, wo ----
            nc.sync.dma_start(wk_sb[:], wk[:])
            nc.sync.dma_start(wv_sb[:], wv[:])
            for k2 in range(4):
                nc.sync.dma_start(
                    x_sb[:, 0, 2 * k2 : 2 * k2 + 2, :], xh0[:, 2 * k2 : 2 * k2 + 2, :]
                )
            for kd in range(8, KD):  # finer tail pacing for K p0's last matmuls
                nc.sync.dma_start(x_sb[:, 0, kd : kd + 1, :], xh0[:, kd : kd + 1, :])
            # cos before wq: K p0's PSUM pair is drained by a cos-gated DVE
            # multiply, and q0 p0's matmuls wait on that ring slot
            nc.sync.dma_start(cos_sb[:], cost[:, :])
            nc.sync.dma_start(wq_sb[:], wq[:])
            for k4 in range(4):
                nc.sync.dma_start(
                    x_sb[:, 1, 4 * k4 : 4 * k4 + 4, :], xh1[:, 4 * k4 : 4 * k4 + 4, :]
                )
            nc.sync.dma_start(wo_sb[:], wo[:])

            def x_rhs(kd, ncx):
                return x_sb[:, ncx // 2, kd, ts(ncx % 2, 512)]

            # ---- projection emitters (1024-wide pieces; piece p covers n cols
            # [p*1024, (p+1)*1024), needing only x chunks 2p, 2p+1) ----
            def proj_piece(w_lhs_slice, piece, dst, mul_cos):
                ps = ps_pool.tile([128, 1024], f32, tag="mm", name="proj_ps")
                for half in range(2):
                    ncx = piece * 2 + half
                    for kd in range(KD):
                        nc.tensor.matmul(
                            ps[:, ts(half, 512)],
                            lhsT=w_lhs_slice(kd),
                            rhs=x_rhs(kd, ncx),
                            start=(kd == 0),
                            stop=(kd == KD - 1),
                        )
                sl = slice(piece * 1024, (piece + 1) * 1024)
                if mul_cos:
                    nc.vector.tensor_mul(dst[:, sl], ps, cos_sb[:, sl])
                else:
                    nc.vector.tensor_copy(dst[:, sl], ps)

            def v_transposes(piece):
                # natural-layout v tiles for the two q4 groups of this piece
                for q4 in (piece * 2, piece * 2 + 1):
                    ps_t = ps_pool.tile([128, 512], bf16, tag="mm", name="vt_ps")
                    for j in range(4):
                        mt = q4 * 4 + j
                        nc.tensor.transpose(
                            ps_t[:, ts(j, 128)], vT_sb[:, ts(mt, 128)], ident_sb[:]
                        )
                    nc.vector.tensor_copy(v_sb[:, ts(q4, 512)], ps_t)

            # K/V/q0 piece 0 (x chunks 0,1 only) before any piece-1 work, so
            # the PE rides just behind the x DMA stream; q1-3 are deferred to
            # the gaps between later heads' attention, which starts attention
            # h0 (and hence the gather stream) ~50us earlier.
            def q_proj(h, piece=None):
                for p in (0, 1) if piece is None else (piece,):
                    proj_piece(
                        lambda kd: wq_sb[:, kd, ts(h, 128)],
                        p,
                        qT_sb[:, h, :],
                        True,
                    )

            proj_piece(lambda kd: wk_sb[:, kd, :], 0, kT_sb, True)
            proj_piece(lambda kd: wv_sb[:, kd, :], 0, vT_sb, False)
            v_transposes(0)
            q_proj(0, piece=0)
            proj_piece(lambda kd: wk_sb[:, kd, :], 1, kT_sb, True)
            proj_piece(lambda kd: wv_sb[:, kd, :], 1, vT_sb, False)
            v_transposes(1)
            q_proj(0, piece=1)

            # ---- collective buffers (head 3 split in two n-halves so its
            # gather starts mid-attention and is off the critical path) ----
            ag_ins = []
            ag_outs = []
            for h in range(G):
                nh = N if h < G - 1 else N // 2
                ag_ins.append(
                    [
                        dram_pool.tile([HD, nh], bf16, tag=f"agi{h}{c}", name=f"agi{h}{c}")
                        for c in range(N // nh)
                    ]
                )
                ag_outs.append(
                    [
                        dram_pool.tile(
                            [HKV * HD, nh], bf16, tag=f"ago{h}{c}", name=f"ago{h}{c}"
                        )
                        for c in range(N // nh)
                    ]
                )

            # leading tiny collective: absorbs cross-core rendezvous skew while
            # the PE is busy with projections, so the first real gather is cheap
            bar_in = dram_pool.tile([1, 128], bf16, tag="bar_in", name="bar_in")
            bar_out = dram_pool.tile([4, 128], bf16, tag="bar_out", name="bar_out")
            nc.gpsimd.collective_compute(
                "AllGather",
                mybir.AluOpType.bypass,
                replica_groups=[[0, 1, 2, 3], [4, 5, 6, 7]],
                ins=[bar_in[:].opt()],
                outs=[bar_out[:].opt()],
            )

            # ---- attention: per (h, ncx): 8 m-tile pairs; scores into a
            # [128,1024] PSUM pair, one 1024-wide exp, bf16 AV matmuls, fp8e5
            # DoubleRow denominator matmul. One flat software pipeline over all
            # 32 pairs of the head (scores run 2 pairs ahead of AV) so there is
            # no PE bubble at (h,ncx) group boundaries. ----
            def gather(h, c):
                nc.gpsimd.collective_compute(
                    "AllGather",
                    mybir.AluOpType.bypass,
                    replica_groups=[[0, 1, 2, 3], [4, 5, 6, 7]],
                    ins=[ag_ins[h][c][:].opt()],
                    outs=[ag_outs[h][c][:].opt()],
                )

            def attention(h, qnext=None):
                accs = {}  # ncx -> (ot_ps, sums_ps)
                sts = {}  # pair index -> (st, st8)
                qp_ps = [None]  # current q-proj psum chunk (dedicated bank)

                def emit_qproj(i):
                    # two contraction steps of head qnext's projection per
                    # iteration: independent PE work that absorbs the
                    # exp->cast handoff latency which otherwise stalls av/sums
                    c = i // 8  # ncx chunk of qT being produced
                    if i % 8 == 0:
                        qp_ps[0] = psacc_pool.tile(
                            [128, 512], f32, tag="qp", name="qp"
                        )
                    for kd in (2 * (i % 8), 2 * (i % 8) + 1):
                        nc.tensor.matmul(
                            qp_ps[0],
                            lhsT=wq_sb[:, kd, ts(qnext, 128)],
                            rhs=x_rhs(kd, c),
                            start=(kd == 0),
                            stop=(kd == KD - 1),
                        )
                    if i % 8 == 7:
                        nc.vector.tensor_mul(
                            qT_sb[:, qnext, ts(c, 512)], qp_ps[0], cos_sb[:, ts(c, 512)]
                        )

                def emit_scores(i):
                    ncx, t = divmod(i, NPAIR)
                    ps = ps_pool.tile([128, 1024], f32, tag="mm", name="sc_ps")
                    for j in range(2):
                        mt = 2 * t + j
                        nc.tensor.matmul(
                            ps[:, ts(j, 512)],
                            lhsT=kT_sb[:, ts(mt, 128)],
                            rhs=qT_sb[:, h, ts(ncx, 512)],
                            start=True,
                            stop=True,
                        )
                    st = st_pool.tile([128, 1024], bf16, tag="st", name="st")
                    nc.scalar.activation(st[:], ps, AFT.Exp, bias=ebias_sb[:])
                    st8 = st_pool.tile([128, 2, 512], f8e5, tag="st8", name="st8")
                    nc.vector.tensor_copy(st8.rearrange("p a b -> p (a b)"), st[:])
                    sts[i] = (st, st8)

                def emit_av(i):
                    ncx, t = divmod(i, NPAIR)
                    if t == 0:
                        accs[ncx] = (
                            psacc_pool.tile(
                                [128, 512], f32, tag="ot", name="ot", bufs=2
                            ),
                            psacc_pool.tile([128, 512], f32, tag="sums", name="sums"),
                        )
                    ot_ps, sums_ps = accs[ncx]
                    st, st8 = sts.pop(i)
                    for j in range(2):
                        mt = 2 * t + j
                        nc.tensor.matmul(
                            ot_ps,
                            lhsT=v_sb[:, ts(mt, 128)],
                            rhs=st[:, ts(j, 512)],
                            start=(t == 0 and j == 0),
                            stop=(t == NPAIR - 1 and j == 1),
                        )
                    nc.tensor.matmul(
                        sums_ps,
                        lhsT=ones8[:],
                        rhs=st8[:],
                        start=(t == 0),
                        stop=(t == NPAIR - 1),
                        perf_mode=mybir.MatmulPerfMode.DoubleRow,
                    )
                    if t == NPAIR - 1:
                        recip_sb = ev_pool.tile([128, 512], f32, tag="recip")
                        nc.vector.reciprocal_approx_fast(recip_sb[:], sums_ps)
                        otn_sb = otn_pool.tile([128, 512], bf16, tag="otn")
                        nc.vector.tensor_mul(otn_sb[:], ot_ps, recip_sb[:])
                        if h < G - 1:
                            nc.sync.dma_start(ag_ins[h][0][:, ts(ncx, 512)], otn_sb[:])
                            if ncx == NC512 - 1:
                                gather(h, 0)
                        else:
                            # head 3: two half-gathers, each issued as soon as
                            # its half of the attention output is ready
                            nc.sync.dma_start(
                                ag_ins[h][ncx // 2][:, ts(ncx % 2, 512)], otn_sb[:]
                            )
                            if ncx % 2 == 1:
                                gather(h, ncx // 2)

                LOOK = 2
                PAIRS = NC512 * NPAIR
                for i in range(PAIRS + LOOK):
                    if i < PAIRS:
                        emit_scores(i)
                        if qnext is not None:
                            emit_qproj(i)
                    if i >= LOOK:
                        emit_av(i - LOOK)

            # each head's attention interleaves the next head's q projection
            # (PE order also defers the x_sb -> outT_acc slot reuse until
            # after q3's projection, which ends inside attention h2)
            attention(0, qnext=1)
            attention(1, qnext=2)
            attention(2, qnext=3)
            attention(3)

            # outT fp32 accumulator for the output projection; reuses x_sb's
            # SBUF slot (x is dead after q3's projection)
            outT_acc = big_pool.tile([128, DC, N], f32, tag="big")

            # ---- per-head out-proj slab accumulation (after all attention) ----
            def slab_chunk(h, ag_sb, dc, np2):
                ps = ps_pool.tile([128, 1024], f32, tag="mm", name=f"sd{h}")
                for half in range(2):
                    nn = np2 * 2 + half
                    for r in range(HKV):
                        jc = r * G + h
                        nc.tensor.matmul(
                            ps[:, ts(half, 512)],
                            lhsT=wo_sb[:, jc, ts(dc, 128)],
                            rhs=ag_sb[:, r, ts(nn, 512)],
                            start=(r == 0),
                            stop=(r == HKV - 1),
                        )
                sl = slice(np2 * 1024, (np2 + 1) * 1024)
                if h == 0:
                    nc.vector.tensor_copy(outT_acc[:, dc, sl], ps)
                else:
                    nc.vector.tensor_add(outT_acc[:, dc, sl], ps, outT_acc[:, dc, sl])

            def slab_contribution(h):
                ag_sb = ag_pool.tile([128, HKV, N], bf16, tag="agsb", name=f"agsb{h}")
                nchunks = len(ag_outs[h])
                cw = N // nchunks
                for c in range(nchunks):
                    ag_v = ag_outs[h][c].rearrange("(r p) n -> p r n", p=128)
                    nc.sync.dma_start(ag_sb[:, :, c * cw : (c + 1) * cw], ag_v[:])
                if h < G - 1:
                    for dc in range(DC):
                        for np2 in range(2):  # 1024-wide n pairs
                            slab_chunk(h, ag_sb, dc, np2)
                else:
                    # last head: sweep n-half-major so the first gathered half
                    # is consumed while the second half's gather is in flight;
                    # ship each final 128-row chunk on alternating DMA queues
                    for np2 in range(2):
                        for dc in range(DC):
                            slab_chunk(h, ag_sb, dc, np2)
                            if np2 == 1:
                                eng = nc.sync if dc % 2 == 0 else nc.scalar
                                eng.dma_start(out[ts(dc, 128), :], outT_acc[:, dc, :])

            for h in range(G):
                slab_contribution(h)

    nc.compile()
    _NC_CACHE["nc"] = nc
    return nc


def kernel(x, Wq, Wk, Wv, Wo):
    _install_axon_ntff_hook()
    import ml_dtypes

    import concourse.bass_utils as bass_utils

    bass_utils.upload_artifacts = lambda tmpdir: str(tmpdir)
    from concourse.bass_utils import run_bass_kernel_spmd

    x = np.asarray(x, dtype=np.float32)
    Wq = np.asarray(Wq, dtype=np.float32)
    Wk = np.asarray(Wk, dtype=np.float32)
    Wv = np.asarray(Wv, dtype=np.float32)
    Wo = np.asarray(Wo, dtype=np.float32)

    bf = ml_dtypes.bfloat16
    scale = np.float32(HD**-0.5)
    wq_f = (_fold_rope(Wq, H) * scale).astype(bf)  # [D, 2048]
    wk_f = _fold_rope(Wk, HKV).astype(bf)  # [D, 512]
    wv_f = Wv.astype(bf)  # [D, 512]
    wo_f = Wo.astype(bf)  # [2048, D]
    cos_t = _cos_table()  # [128, N] fp32

    KD = D // 128

    def pack(w):  # [D, cols] -> [128, KD, cols] partition-major (big DMA lines)
        return np.ascontiguousarray(
            w.reshape(KD, 128, w.shape[1]).transpose(1, 0, 2)
        )

    xh = []  # xh[b][c]: [128, KD, 1024]
    for b in range(B):
        xt_r = np.ascontiguousarray(x[b].T).astype(bf).reshape(KD, 128, 2, 1024)
        xh.append(
            [np.ascontiguousarray(xt_r[:, :, c, :].transpose(1, 0, 2)) for c in (0, 1)]
        )

    in_maps = []
    for c in range(N_CORES):
        b, g = divmod(c, HKV)
        in_maps.append(
            {
                "xh0": xh[b][0],
                "xh1": xh[b][1],
                "wq": pack(wq_f[:, g * JL : (g + 1) * JL]),
                "wk": pack(wk_f[:, g * HD : (g + 1) * HD]),
                "wv": pack(wv_f[:, g * HD : (g + 1) * HD]),
                "wo": pack(wo_f[:, g * DSLICE : (g + 1) * DSLICE]),
                "cost": cos_t,
            }
        )

    nc = _build_nc()
    res = run_bass_kernel_spmd(nc, in_maps, list(range(N_CORES)))

    out = np.empty((B, N, D), dtype=np.float32)
    for c in range(N_CORES):
        b, g = divmod(c, HKV)
        out[b, :, g * DSLICE : (g + 1) * DSLICE] = res.results[c]["out"].T
    return out


# revision 29
# speedup vs baseline: 1.0163x; 1.0163x over previous
"""GroupedQueryAttention (B=2, N=2048, D=2048, H=16, HKV=4, HD=128) on 8 trn2 cores.

Sharding: core c handles (batch b = c//4, kv-head g = c%4): 4 q-heads + 1 kv head.
RoPE (with the reference's sin==cos quirk) is folded into Wq/Wk host-side,
so on-device RoPE is an elementwise multiply by a precomputed cos table.
Per-head AllGather of normalized attention outputs across the 4 cores of a
batch, then each core accumulates its 512 Wo columns over all 16 heads in a
transposed [d, n] fp32 SBUF accumulator (host transposes back).

Performance structure (vs the 467us v1 baseline; now ~370us):
  - Softmax-denominator matmuls run in fp8e5 + DoubleRow perf mode (two
    m-tiles contracted per pass at 0.5 cyc/row): ~4x less PE time than the
    bf16 ones-matmul. exp carries a -2*ln2 bias (applied on the ScalarE
    activation) so exp values stay well under the e5m2 max; the scale cancels
    exactly in the P/sum ratio. AV stays bf16 (fp8 P measurably breaks the
    2e-2 error budget; the all-positive denominator sum averages fp8 noise
    away).
  - exp runs on 1024-wide PSUM score pairs (two m-tiles per ACTIVATE),
    halving ScalarE instruction overhead; a DVE copy casts each pair to fp8
    for the denominator matmul.
  - Attention is one flat software pipeline per head over all 32 (ncx, pair)
    steps with scores emitted 2 pairs ahead of AV, so the PE has no bubble at
    group boundaries (PSUM: 3x[128,1024] scores ring + ot + sums = 8 banks).
  - Emission order starts attention h0 right after K, V, q0 projections;
    head h's attention interleaves head h+1's q projection (2 matmuls per
    pair), giving the PE independent work while the exp->cast chain settles.
    This launches the AllGather stream ~50us earlier (the 4 gathers are the
    near-critical path once attention is fast). Head 3's gather is split in
    two n-halves issued mid-attention so the last slab never stalls on it.
  - All inputs are host-prepacked partition-major so DMA lines are 8-32KB
    (~270GB/s vs ~170GB/s for row-major views); x arrives as two n-halves
    and K proj paces with the first half's sub-chunks.
  - Final out DMAs issue per-128-row chunk as the last slab completes them,
    alternating the two HWDGE queues (SP/Activation).
"""

import math
import sys
import types

import numpy as np

B, N, D = 2, 2048, 2048
H, HKV, HD = 16, 4, 128
G = H // HKV  # q heads per kv head = 4
N_CORES = 8
ROPE_BASE = 10000.0
DSLICE = D // G  # 512 output columns per core
JL = G * HD  # 512 local attention-output rows per core
EXP_BIAS = -2.0 * math.log(2.0)  # scales all exp values by 1/4 (cancels in ratio)


def _install_axon_ntff_hook():
    """This container's antenv lacks axon_hooks; inject it so trace=True works."""
    if "antenv.axon_hooks" in sys.modules:
        return
    try:
        from trn_agent_boot.trn_boot import _ntff_profile_via_ctypes

        hook = _ntff_profile_via_ctypes("/opt/axon/libaxon_pjrt.so")
    except Exception:
        hook = None
    mod = types.ModuleType("antenv.axon_hooks")
    mod.get_axon_ntff_profile_hook = lambda: hook
    mod.set_axon_ntff_profile_hook = lambda h: None
    sys.modules["antenv.axon_hooks"] = mod


def _fold_rope(w: np.ndarray, n_heads: int) -> np.ndarray:
    """Return W' with the (sin==cos) RoPE mixing folded in: x@W' = M(x@W) per head."""
    wf = w.reshape(D, n_heads, HD)
    lo, hi = wf[..., : HD // 2], wf[..., HD // 2 :]
    return np.concatenate([lo - hi, hi + lo], axis=-1).reshape(D, n_heads * HD)


def _cos_table() -> np.ndarray:
    inv_freq = 1.0 / (ROPE_BASE ** (np.arange(0, HD, 2, dtype=np.float64) / HD))
    freqs = np.arange(N, dtype=np.float64)[:, None] * inv_freq[None, :]  # [N, 64]
    emb = np.concatenate([freqs, freqs], axis=-1)  # [N, 128]
    return np.cos(emb).T.astype(np.float32).copy()  # [128, N]


_NC_CACHE: dict = {}


def _build_nc():
    if "nc" in _NC_CACHE:
        return _NC_CACHE["nc"]

    import concourse.bacc as bacc
    import concourse.mybir as mybir
    import concourse.tile as tile
    from concourse.bass import ts
    from concourse.masks import make_identity

    f32 = mybir.dt.float32
    bf16 = mybir.dt.bfloat16
    f8e5 = mybir.dt.float8e5
    AFT = mybir.ActivationFunctionType
    KD = D // 128  # 16 contraction chunks
    NT = N // 128  # 16 m tiles of 128
    NC512 = N // 512  # 4 n chunks of 512
    NPAIR = NT // 2  # 8 m-tile pairs
    DC = DSLICE // 128  # 4 output-row chunks of 128

    nc = bacc.Bacc(target_bir_lowering=False, debug=False, num_devices=N_CORES)

    # all inputs host-prepacked partition-major so DMA lines are 8-32KB
    # (row-major [D, cols] views only give 1-2KB lines and ~170GB/s; packed
    # transfers sustain ~270+GB/s). x comes as two n-halves so K/V/q0
    # projections of the first 1024 columns start ~20us earlier.
    xh0 = nc.dram_tensor("xh0", [128, KD, 1024], bf16, kind="ExternalInput")
    xh1 = nc.dram_tensor("xh1", [128, KD, 1024], bf16, kind="ExternalInput")
    wq = nc.dram_tensor("wq", [128, KD, JL], bf16, kind="ExternalInput")
    wk = nc.dram_tensor("wk", [128, KD, HD], bf16, kind="ExternalInput")
    wv = nc.dram_tensor("wv", [128, KD, HD], bf16, kind="ExternalInput")
    wo = nc.dram_tensor("wo", [128, KD, DSLICE], bf16, kind="ExternalInput")
    cost = nc.dram_tensor("cost", [HD, N], f32, kind="ExternalInput")
    # transposed output: outT[d, n]; host transposes back
    out = nc.dram_tensor("out", [DSLICE, N], f32, kind="ExternalOutput")

    with tile.TileContext(nc) as tc:
        with (
            tc.tile_pool(name="big", bufs=1) as big_pool,
            tc.tile_pool(name="ag", bufs=3) as ag_pool,
            tc.tile_pool(name="otn", bufs=6) as otn_pool,
            tc.tile_pool(name="wpool", bufs=1) as w_pool,
            tc.tile_pool(name="work", bufs=1) as work_pool,
            tc.tile_pool(name="st", bufs=3) as st_pool,
            tc.tile_pool(name="ev", bufs=2) as ev_pool,
            tc.tile_pool(name="psmm", bufs=2, space="PSUM") as ps_pool,
            tc.tile_pool(name="psacc", bufs=1, space="PSUM") as psacc_pool,
            tc.tile_pool(name="dram", bufs=1, space="DRAM") as dram_pool,
        ):
            # ---- persistent SBUF tensors ----
            x_sb = big_pool.tile([128, 2, KD, 1024], bf16, tag="big")
            wq_sb = w_pool.tile([128, KD, JL], bf16, tag="wq")
            wk_sb = w_pool.tile([128, KD, HD], bf16, tag="wk")
            wv_sb = w_pool.tile([128, KD, HD], bf16, tag="wv")
            wo_sb = w_pool.tile([128, KD, DSLICE], bf16, tag="wo")
            cos_sb = w_pool.tile([128, N], f32, tag="cos")
            qT_sb = work_pool.tile([128, G, N], bf16, tag="qT")
            kT_sb = work_pool.tile([128, N], bf16, tag="kT")
            vT_sb = ag_pool.tile([128, HKV, N], bf16, tag="agsb", name="vT_sb")[:, 0, :]
            v_sb = work_pool.tile([128, N], bf16, tag="v")  # [m-part, mt*128+hd]
            ones8 = work_pool.tile([128, 2, 128], f8e5, tag="ones8")
            ident_sb = work_pool.tile([128, 128], bf16, tag="ident")
            ebias_sb = work_pool.tile([128, 1], f32, tag="ebias")

            nc.gpsimd.memset(ones8[:], 1.0)
            nc.gpsimd.memset(ebias_sb[:], EXP_BIAS)
            make_identity(nc, ident_sb[:])

            # ---- input DMAs: weights for K/V first, then x half 0 in 4-kd
            # sub-chunks (K p0 starts paced with them), then wq, x half 1, wo ----
            nc.sync.dma_start(wk_sb[:], wk[:])
            nc.sync.dma_start(wv_sb[:], wv[:])
            for k2 in range(8):
                nc.sync.dma_start(
                    x_sb[:, 0, 2 * k2 : 2 * k2 + 2, :], xh0[:, 2 * k2 : 2 * k2 + 2, :]
                )
            nc.sync.dma_start(wq_sb[:], wq[:])
            nc.sync.dma_start(cos_sb[:], cost[:, :])
            for k4 in range(4):
                nc.sync.dma_start(
                    x_sb[:, 1, 4 * k4 : 4 * k4 + 4, :], xh1[:, 4 * k4 : 4 * k4 + 4, :]
                )
            nc.sync.dma_start(wo_sb[:], wo[:])

            def x_rhs(kd, ncx):
                return x_sb[:, ncx // 2, kd, ts(ncx % 2, 512)]

            # ---- projection emitters (1024-wide pieces; piece p covers n cols
            # [p*1024, (p+1)*1024), needing only x chunks 2p, 2p+1) ----
            def proj_piece(w_lhs_slice, piece, dst, mul_cos):
                ps = ps_pool.tile([128, 1024], f32, tag="mm", name="proj_ps")
                for half in range(2):
                    ncx = piece * 2 + half
                    for kd in range(KD):
                        nc.tensor.matmul(
                            ps[:, ts(half, 512)],
                            lhsT=w_lhs_slice(kd),
                            rhs=x_rhs(kd, ncx),
                            start=(kd == 0),
                            stop=(kd == KD - 1),
                        )
                sl = slice(piece * 1024, (piece + 1) * 1024)
                if mul_cos:
                    nc.vector.tensor_mul(dst[:, sl], ps, cos_sb[:, sl])
                else:
                    nc.vector.tensor_copy(dst[:, sl], ps)

            def v_transposes(piece):
                # natural-layout v tiles for the two q4 groups of this piece
                for q4 in (piece * 2, piece * 2 + 1):
                    ps_t = ps_pool.tile([128, 512], bf16, tag="mm", name="vt_ps")
                    for j in range(4):
                        mt = q4 * 4 + j
                        nc.tensor.transpose(
                            ps_t[:, ts(j, 128)], vT_sb[:, ts(mt, 128)], ident_sb[:]
                        )
                    nc.vector.tensor_copy(v_sb[:, ts(q4, 512)], ps_t)

            # K/V/q0 piece 0 (x chunks 0,1 only) before any piece-1 work, so
            # the PE rides just behind the x DMA stream; q1-3 are deferred to
            # the gaps between later heads' attention, which starts attention
            # h0 (and hence the gather stream) ~50us earlier.
            def q_proj(h, piece=None):
                for p in (0, 1) if piece is None else (piece,):
                    proj_piece(
                        lambda kd: wq_sb[:, kd, ts(h, 128)],
                        p,
                        qT_sb[:, h, :],
                        True,
                    )

            proj_piece(lambda kd: wk_sb[:, kd, :], 0, kT_sb, True)
            proj_piece(lambda kd: wv_sb[:, kd, :], 0, vT_sb, False)
            v_transposes(0)
            q_proj(0, piece=0)
            proj_piece(lambda kd: wk_sb[:, kd, :], 1, kT_sb, True)
            proj_piece(lambda kd: wv_sb[:, kd, :], 1, vT_sb, False)
            v_transposes(1)
            q_proj(0, piece=1)

            # ---- collective buffers (head 3 split in two n-halves so its
            # gather starts mid-attention and is off the critical path) ----
            ag_ins = []
            ag_outs = []
            for h in range(G):
                nh = N if h < G - 1 else N // 2
                ag_ins.append(
                    [
                        dram_pool.tile([HD, nh], bf16, tag=f"agi{h}{c}", name=f"agi{h}{c}")
                        for c in range(N // nh)
                    ]
                )
                ag_outs.append(
                    [
                        dram_pool.tile(
                            [HKV * HD, nh], bf16, tag=f"ago{h}{c}", name=f"ago{h}{c}"
                        )
                        for c in range(N // nh)
                    ]
                )

            # leading tiny collective: absorbs cross-core rendezvous skew while
            # the PE is busy with projections, so the first real gather is cheap
            bar_in = dram_pool.tile([1, 128], bf16, tag="bar_in", name="bar_in")
            bar_out = dram_pool.tile([4, 128], bf16, tag="bar_out", name="bar_out")
            nc.gpsimd.collective_compute(
                "AllGather",
                mybir.AluOpType.bypass,
                replica_groups=[[0, 1, 2, 3], [4, 5, 6, 7]],
                ins=[bar_in[:].opt()],
                outs=[bar_out[:].opt()],
            )

            # ---- attention: per (h, ncx): 8 m-tile pairs; scores into a
            # [128,1024] PSUM pair, one 1024-wide exp, bf16 AV matmuls, fp8e5
            # DoubleRow denominator matmul. One flat software pipeline over all
            # 32 pairs of the head (scores run 2 pairs ahead of AV) so there is
            # no PE bubble at (h,ncx) group boundaries. ----
            def gather(h, c):
                nc.gpsimd.collective_compute(
                    "AllGather",
                    mybir.AluOpType.bypass,
                    replica_groups=[[0, 1, 2, 3], [4, 5, 6, 7]],
                    ins=[ag_ins[h][c][:].opt()],
                    outs=[ag_outs[h][c][:].opt()],
                )

            def attention(h, qnext=None):
                accs = {}  # ncx -> (ot_ps, sums_ps)
                sts = {}  # pair index -> (st, st8)
                qp_ps = [None]  # current q-proj psum chunk (dedicated bank)

                def emit_qproj(i):
                    # two contraction steps of head qnext's projection per
                    # iteration: independent PE work that absorbs the
                    # exp->cast handoff latency which otherwise stalls av/sums
                    c = i // 8  # ncx chunk of qT being produced
                    if i % 8 == 0:
                        qp_ps[0] = psacc_pool.tile(
                            [128, 512], f32, tag="qp", name="qp"
                        )
                    for kd in (2 * (i % 8), 2 * (i % 8) + 1):
                        nc.tensor.matmul(
                            qp_ps[0],
                            lhsT=wq_sb[:, kd, ts(qnext, 128)],
                            rhs=x_rhs(kd, c),
                            start=(kd == 0),
                            stop=(kd == KD - 1),
                        )
                    if i % 8 == 7:
                        nc.vector.tensor_mul(
                            qT_sb[:, qnext, ts(c, 512)], qp_ps[0], cos_sb[:, ts(c, 512)]
                        )

                def emit_scores(i):
                    ncx, t = divmod(i, NPAIR)
                    ps = ps_pool.tile([128, 1024], f32, tag="mm", name="sc_ps")
                    for j in range(2):
                        mt = 2 * t + j
                        nc.tensor.matmul(
                            ps[:, ts(j, 512)],
                            lhsT=kT_sb[:, ts(mt, 128)],
                            rhs=qT_sb[:, h, ts(ncx, 512)],
                            start=True,
                            stop=True,
                        )
                    st = st_pool.tile([128, 1024], bf16, tag="st", name="st")
                    nc.scalar.activation(st[:], ps, AFT.Exp, bias=ebias_sb[:])
                    st8 = st_pool.tile([128, 2, 512], f8e5, tag="st8", name="st8")
                    nc.vector.tensor_copy(st8.rearrange("p a b -> p (a b)"), st[:])
                    sts[i] = (st, st8)

                def emit_av(i):
                    ncx, t = divmod(i, NPAIR)
                    if t == 0:
                        accs[ncx] = (
                            psacc_pool.tile(
                                [128, 512], f32, tag="ot", name="ot", bufs=2
                            ),
                            psacc_pool.tile([128, 512], f32, tag="sums", name="sums"),
                        )
                    ot_ps, sums_ps = accs[ncx]
                    st, st8 = sts.pop(i)
                    for j in range(2):
                        mt = 2 * t + j
                        nc.tensor.matmul(
                            ot_ps,
                            lhsT=v_sb[:, ts(mt, 128)],
                            rhs=st[:, ts(j, 512)],
                            start=(t == 0 and j == 0),
                            stop=(t == NPAIR - 1 and j == 1),
                        )
                    nc.tensor.matmul(
                        sums_ps,
                        lhsT=ones8[:],
                        rhs=st8[:],
                        start=(t == 0),
                        stop=(t == NPAIR - 1),
                        perf_mode=mybir.MatmulPerfMode.DoubleRow,
                    )
                    if t == NPAIR - 1:
                        recip_sb = ev_pool.tile([128, 512], f32, tag="recip")
                        nc.vector.reciprocal_approx_fast(recip_sb[:], sums_ps)
                        otn_sb = otn_pool.tile([128, 512], bf16, tag="otn")
                        nc.vector.tensor_mul(otn_sb[:], ot_ps, recip_sb[:])
                        if h < G - 1:
                            nc.sync.dma_start(ag_ins[h][0][:, ts(ncx, 512)], otn_sb[:])
                            if ncx == NC512 - 1:
                                gather(h, 0)
                        else:
                            # head 3: two half-gathers, each issued as soon as
                            # its half of the attention output is ready
                            nc.sync.dma_start(
                                ag_ins[h][ncx // 2][:, ts(ncx % 2, 512)], otn_sb[:]
                            )
                            if ncx % 2 == 1:
                                gather(h, ncx // 2)

                LOOK = 2
                PAIRS = NC512 * NPAIR
                for i in range(PAIRS + LOOK):
                    if i < PAIRS:
                        emit_scores(i)
                        if qnext is not None:
                            emit_qproj(i)
                    if i >= LOOK:
                        emit_av(i - LOOK)

            # each head's attention interleaves the next head's q projection
            # (PE order also defers the x_sb -> outT_acc slot reuse until
            # after q3's projection, which ends inside attention h2)
            attention(0, qnext=1)
            attention(1, qnext=2)
            attention(2, qnext=3)
            attention(3)

            # outT fp32 accumulator for the output projection; reuses x_sb's
            # SBUF slot (x is dead after q3's projection)
            outT_acc = big_pool.tile([128, DC, N], f32, tag="big")

            # ---- per-head out-proj slab accumulation (after all attention) ----
            def slab_chunk(h, ag_sb, dc, np2):
                ps = ps_pool.tile([128, 1024], f32, tag="mm", name=f"sd{h}")
                for half in range(2):
                    nn = np2 * 2 + half
                    for r in range(HKV):
                        jc = r * G + h
                        nc.tensor.matmul(
                            ps[:, ts(half, 512)],
                            lhsT=wo_sb[:, jc, ts(dc, 128)],
                            rhs=ag_sb[:, r, ts(nn, 512)],
                            start=(r == 0),
                            stop=(r == HKV - 1),
                        )
                sl = slice(np2 * 1024, (np2 + 1) * 1024)
                if h == 0:
                    nc.vector.tensor_copy(outT_acc[:, dc, sl], ps)
                else:
                    nc.vector.tensor_add(outT_acc[:, dc, sl], ps, outT_acc[:, dc, sl])

            def slab_contribution(h):
                ag_sb = ag_pool.tile([128, HKV, N], bf16, tag="agsb", name=f"agsb{h}")
                nchunks = len(ag_outs[h])
                cw = N // nchunks
                for c in range(nchunks):
                    ag_v = ag_outs[h][c].rearrange("(r p) n -> p r n", p=128)
                    nc.sync.dma_start(ag_sb[:, :, c * cw : (c + 1) * cw], ag_v[:])
                if h < G - 1:
                    for dc in range(DC):
                        for np2 in range(2):  # 1024-wide n pairs
                            slab_chunk(h, ag_sb, dc, np2)
                else:
                    # last head: sweep n-half-major so the first gathered half
                    # is consumed while the second half's gather is in flight;
                    # ship each final 128-row chunk on alternating DMA queues
                    for np2 in range(2):
                        for dc in range(DC):
                            slab_chunk(h, ag_sb, dc, np2)
                            if np2 == 1:
                                eng = nc.sync if dc % 2 == 0 else nc.scalar
                                eng.dma_start(out[ts(dc, 128), :], outT_acc[:, dc, :])

            for h in range(G):
                slab_contribution(h)

    nc.compile()
    _NC_CACHE["nc"] = nc
    return nc


def kernel(x, Wq, Wk, Wv, Wo):
    _install_axon_ntff_hook()
    import ml_dtypes

    import concourse.bass_utils as bass_utils

    bass_utils.upload_artifacts = lambda tmpdir: str(tmpdir)
    from concourse.bass_utils import run_bass_kernel_spmd

    x = np.asarray(x, dtype=np.float32)
    Wq = np.asarray(Wq, dtype=np.float32)
    Wk = np.asarray(Wk, dtype=np.float32)
    Wv = np.asarray(Wv, dtype=np.float32)
    Wo = np.asarray(Wo, dtype=np.float32)

    bf = ml_dtypes.bfloat16
    scale = np.float32(HD**-0.5)
    wq_f = (_fold_rope(Wq, H) * scale).astype(bf)  # [D, 2048]
    wk_f = _fold_rope(Wk, HKV).astype(bf)  # [D, 512]
    wv_f = Wv.astype(bf)  # [D, 512]
    wo_f = Wo.astype(bf)  # [2048, D]
    cos_t = _cos_table()  # [128, N] fp32

    KD = D // 128

    def pack(w):  # [D, cols] -> [128, KD, cols] partition-major (big DMA lines)
        return np.ascontiguousarray(
            w.reshape(KD, 128, w.shape[1]).transpose(1, 0, 2)
        )

    xh = []  # xh[b][c]: [128, KD, 1024]
    for b in range(B):
        xt_r = np.ascontiguousarray(x[b].T).astype(bf).reshape(KD, 128, 2, 1024)
        xh.append(
            [np.ascontiguousarray(xt_r[:, :, c, :].transpose(1, 0, 2)) for c in (0, 1)]
        )

    in_maps = []
    for c in range(N_CORES):
        b, g = divmod(c, HKV)
        in_maps.append(
            {
                "xh0": xh[b][0],
                "xh1": xh[b][1],
                "wq": pack(wq_f[:, g * JL : (g + 1) * JL]),
                "wk": pack(wk_f[:, g * HD : (g + 1) * HD]),
                "wv": pack(wv_f[:, g * HD : (g + 1) * HD]),
                "wo": pack(wo_f[:, g * DSLICE : (g + 1) * DSLICE]),
                "cost": cos_t,
            }
        )

    nc = _build_nc()
    res = run_bass_kernel_spmd(nc, in_maps, list(range(N_CORES)))

    out = np.empty((B, N, D), dtype=np.float32)
    for c in range(N_CORES):
        b, g = divmod(c, HKV)
        out[b, :, g * DSLICE : (g + 1) * DSLICE] = res.results[c]["out"].T
    return out
